# revision 21
# baseline (speedup 1.0000x reference)
"""Multi-head attention (B=2, L=2048, D=1024, H=16) on 8 TRN2 NeuronCores.

Sharding: core c handles batch b = c//4 and head group g = c%4 (4 heads,
256 features). Per core:
  - 8 warmup matmuls at t=0 get the PE HAM clock warm during the input DMA
  - project q, k (feature-major, ft-interleaved d-chains) and v (row-major)
    in bf16; k/v inputs on the sync DMA queue, q inputs on the gpsimd queue;
    bias adds alternate DVE/ScalarE so the pool transition never stalls PE
  - attention emitted in runs-of-2 slots to minimize PE tile-config thrash:
    [scores m, m+1] [den m-3, m-2] [ctx m-5, m-4]
      scores^T per m: both heads in one [128,1024] PSUM tile (row-tiled pair)
      exp: tile-granular split - 5 of 16 m-slots on VectorE (Schraudolph
      int16 bit-trick), the rest on ScalarE table exp
      ctx chains: col-tiled pairs, 2 heads share one [128,512] PSUM bank
      den: M=1 ones-matmul chains at col positions 0/32/64/96
      normalize: deferred into the next h-iter's first slots; den rows are
      copied from PSUM, reciprocated, broadcast via a 33-row selector
      matmul, and multiplied into ctxT
  - output projection out^T = Wo_h ctx^T drained into later m-loops;
    the last h-iter uses tighter lags + engine-split exp to shrink the tail
Host sums the 4 per-head-group partials per batch and adds bo.
"""

import math
import os
import sys

sys.path.insert(0, "/opt/trn_rl_repo")

import ml_dtypes
import numpy as np

import concourse.bass as bass
import concourse.mybir as mybir
import concourse.tile as tile
from concourse import bacc
from concourse.bass_utils import run_bass_kernel_spmd

B, L, D, H, DH = 2, 2048, 1024, 16, 64
NCORES = 8
HPC = 4                  # heads per core
FPC = HPC * DH           # 256 features per core
ND = D // 128            # 8 contraction tiles
NFT = FPC // 128         # 2 feature tiles for q/k/ctx
NM = L // 128            # 16 key tiles
SCALE = 1.0 / math.sqrt(DH)
CDT = mybir.dt.bfloat16
NP_CDT = ml_dtypes.bfloat16
F32 = mybir.dt.float32
I16 = mybir.dt.int16
F16 = mybir.dt.float16
EXP = mybir.ActivationFunctionType.Exp
MULT = mybir.AluOpType.mult
ADD = mybir.AluOpType.add
OUT_NAME = "outT"

# Schraudolph exp in bf16 bit-space: bf16(i16(x*A16 + B16)) ~ exp(x*SCALE)
LOG2E = 1.4426950408889634
A16 = float(np.float32(SCALE * 128.0 * LOG2E))
B16 = float(np.float32((127 << 7) - 5.5908))

DVE_SET = frozenset(
    int(x) for x in os.environ.get("DVE_SET", "3,6,9,12,15").split(",") if x
)  # m-slots whose exp runs on VectorE (Schraudolph); rest on ScalarE

_CACHE = {}


def build_nc():
    nc = bacc.Bacc(
        "TRN2",
        target_bir_lowering=False,
        debug=False,
        enable_asserts=False,
        num_devices=NCORES,
    )
    xqT_d = nc.dram_tensor("xqT", [D, L], CDT, kind="ExternalInput")
    xkT_d = nc.dram_tensor("xkT", [D, L], CDT, kind="ExternalInput")
    xvT_d = nc.dram_tensor("xvT", [D, L], CDT, kind="ExternalInput")
    wq_d = nc.dram_tensor("wqT", [D, FPC], CDT, kind="ExternalInput")
    wk_d = nc.dram_tensor("wkT", [D, FPC], CDT, kind="ExternalInput")
    wv_d = nc.dram_tensor("wvT", [D, FPC], CDT, kind="ExternalInput")
    wo_d = nc.dram_tensor("woT", [FPC, D], CDT, kind="ExternalInput")
    bq_d = nc.dram_tensor("bq2", [128, NFT], F32, kind="ExternalInput")
    bk_d = nc.dram_tensor("bk2", [128, NFT], F32, kind="ExternalInput")
    bvb_d = nc.dram_tensor("bvb", [128, FPC], F32, kind="ExternalInput")
    out_d = nc.dram_tensor(OUT_NAME, [D, L], F16, kind="ExternalOutput")

    with tile.TileContext(nc) as tc:
        with tc.tile_pool(name="persist", bufs=1) as pp:
            qT = pp.tile([128, NFT, L], CDT)
            kT = pp.tile([128, NFT, L], CDT)
            vsb = pp.tile([128, NM, FPC], CDT)
            ctxT = pp.tile([128, NFT, L], CDT)
            wo_sb = pp.tile([128, NFT, D], CDT)
            bq_sb = pp.tile([128, NFT], F32)
            bk_sb = pp.tile([128, NFT], F32)
            bvb_sb = pp.tile([128, FPC], F32)
            ones_sb = pp.tile([128, 1], CDT)
            sel33 = pp.tile([33, 128], F32)
            wscr = pp.tile([128, 512], CDT)

            nc.vector.memset(wscr[:], 0.0)
            nc.vector.memset(ones_sb[:], 1.0)
            nc.vector.memset(sel33[:], 0.0)
            nc.vector.memset(sel33[0:1, 0:64], 1.0)
            nc.vector.memset(sel33[32:33, 64:128], 1.0)

            if True:
                wq_sb = pp.tile([128, ND, FPC], CDT)
                wk_sb = pp.tile([128, ND, FPC], CDT)
                wv_sb = pp.tile([128, ND, FPC], CDT)
                xq_sb = pp.tile([128, ND, L], CDT)
                xk_sb = pp.tile([128, ND, L], CDT)
                xv_sb = pp.tile([128, ND, L], CDT)
                xq_r = xqT_d.rearrange("(n p) l -> p n l", p=128)
                xk_r = xkT_d.rearrange("(n p) l -> p n l", p=128)
                xv_r = xvT_d.rearrange("(n p) l -> p n l", p=128)
                wq_r = wq_d.rearrange("(n p) f -> p n f", p=128)
                wk_r = wk_d.rearrange("(n p) f -> p n f", p=128)
                wv_r = wv_d.rearrange("(n p) f -> p n f", p=128)
                # both queues carry each stream in sequence so every x
                # tensor gets the full fabric bandwidth while PE consumes it:
                # xk first (k-proj), then xq, then xv; small tensors up front
                # sync queue: k inputs then v inputs; gpsimd queue in
                # parallel: q inputs, then wo + biases
                nc.sync.dma_start(wk_sb[:], wk_r[:])
                for d in range(ND):
                    nc.sync.dma_start(xk_sb[:, d, :], xk_r[:, d, :])
                nc.sync.dma_start(wv_sb[:], wv_r[:])
                for d in range(ND):
                    nc.sync.dma_start(xv_sb[:, d, :], xv_r[:, d, :])
                nc.gpsimd.dma_start(bq_sb[:], bq_d[:])
                nc.gpsimd.dma_start(bk_sb[:], bk_d[:])
                nc.gpsimd.dma_start(wq_sb[:], wq_r[:])
                for d in range(ND):
                    nc.gpsimd.dma_start(xq_sb[:, d, :], xq_r[:, d, :])
                nc.gpsimd.dma_start(bvb_sb[:], bvb_d[:])
                nc.gpsimd.dma_start(
                    wo_sb[:], wo_d.rearrange("(n p) f -> p n f", p=128)
                )

                with tc.tile_pool(name="psA", bufs=8, space="PSUM") as psA:
                    wps = psA.tile([128, 512], F32, tag="projqk", name="warm")
                    for _ in range(8):
                        nc.tensor.matmul(
                            wps[0:1, :], ones_sb[:], wscr[:],
                            start=True, stop=True,
                        )

                    def proj_qk(x_sb, w_sb, b_sb, dstT, ti):
                        # ft-interleaved d-chains: consume x[d] with 8
                        # matmuls per arrival; ldweights fillers bridge the
                        # DMA-paced gaps on the k pass
                        pss = {
                            (ft, ch): psA.tile(
                                [128, 512], F32, tag="projqk",
                                name=f"pjk_{ti}_{ft}_{ch}",
                            )
                            for ft in range(NFT)
                            for ch in range(4)
                        }
                        for d in range(ND):
                            for ft in range(NFT):
                                for ch in range(4):
                                    nc.tensor.matmul(
                                        pss[ft, ch][:],
                                        w_sb[:, d, ft * 128 : (ft + 1) * 128],
                                        x_sb[:, d, ch * 512 : (ch + 1) * 512],
                                        start=(d == 0),
                                        stop=(d == ND - 1),
                                    )
                        for ft in range(NFT):
                            for ch in range(4):
                                if ch % 2 == 0:
                                    nc.vector.tensor_scalar_add(
                                        dstT[:, ft, ch * 512 : (ch + 1) * 512],
                                        pss[ft, ch][:],
                                        b_sb[:, ft : ft + 1],
                                    )
                                else:
                                    nc.scalar.add(
                                        dstT[:, ft, ch * 512 : (ch + 1) * 512],
                                        pss[ft, ch][:],
                                        b_sb[:, ft : ft + 1],
                                    )

                    proj_qk(xk_sb, wk_sb, bk_sb, kT, 1)
                    proj_qk(xq_sb, wq_sb, bq_sb, qT, 0)

                def make_vproj(pool):
                    # v projection kt-chains: row-major [keys, 256 feats]
                    def vproj(kt):
                        ops = pool.tile(
                            [128, 512], F32, tag="acc512", name=f"vp_{kt}"
                        )
                        for d in range(ND):
                            nc.tensor.matmul(
                                ops[:, 0:FPC],
                                xv_sb[:, d, kt * 128 : (kt + 1) * 128],
                                wv_sb[:, d, :],
                                start=(d == 0),
                                stop=(d == ND - 1),
                            )
                        nc.vector.tensor_add(
                            vsb[:, kt, :], ops[:, 0:FPC], bvb_sb[:]
                        )
                    return vproj

                phase_b(
                    nc, tc, qT, kT, vsb, ctxT, wo_sb, ones_sb, sel33,
                    out_d, make_vproj,
                )
    nc.compile()
    return nc


def phase_b(
    nc, tc, qT, kT, vsb, ctxT, wo_sb, ones_sb, sel33, out_d, make_vproj,
):
    with (
        tc.tile_pool(name="sbB", bufs=3) as sm,
        tc.tile_pool(name="psB", bufs=1, space="PSUM") as psB,
    ):
        pb = sm
        carried = []   # deferred output-projection work items (closures)
        deferred = []  # deferred normalize closures from the previous h-iter
        vproj = make_vproj(psB)

        def emit_outproj_lc(lc):
            # out^T chunk [128, 512] per ft8; contract ctxT over NFT
            for ft8 in range(D // 128):
                def work(lc=lc, ft8=ft8, tag="acc512"):
                    ops = psB.tile(
                        [128, 512], F32, tag=tag, name=f"op_{lc}_{ft8}",
                        bufs=1 if tag != "ctx" else 2,
                    )
                    for d2 in range(NFT):
                        nc.tensor.matmul(
                            ops[:],
                            wo_sb[:, d2, ft8 * 128 : (ft8 + 1) * 128],
                            ctxT[:, d2, lc * 512 : (lc + 1) * 512],
                            start=(d2 == 0),
                            stop=(d2 == NFT - 1),
                        )
                    st = sm.tile(
                        [128, 512], F16, tag="ost", bufs=4,
                        name=f"st_{lc}_{ft8}",
                    )
                    if ft8 % 2 == 0:
                        nc.vector.tensor_copy(st[:], ops[:])
                    else:
                        nc.scalar.copy(st[:], ops[:])
                    nc.sync.dma_start(
                        out_d[
                            ft8 * 128 : (ft8 + 1) * 128,
                            lc * 512 : (lc + 1) * 512,
                        ],
                        st[:],
                    )
                carried.append(work)

        first = True
        for qh in range(2):
            for hp in range(2):
                den = psB.tile([128, 512], F32, tag="den", bufs=1,
                               name=f"den_{qh}_{hp}")
                if qh == 0 and hp == 0:
                    # sanitize once: rows besides 0/32/64/96 stay 1.0 so the
                    # direct-PSUM-window reciprocal never sees garbage (psD
                    # bufs=1 keeps the same bank across groups)
                    nc.vector.memset(den[:], 1.0)
                g0 = hp * 2  # head index base within this core's 4 heads
                for h in range(2):
                    last_h = qh == 1 and hp == 1 and h == 1
                    ctx = psB.tile(
                        [128, 512], F32, tag="ctx", bufs=2,
                        name=f"ctx_{qh}_{hp}_{h}",
                    )
                    q0 = qh * 1024 + h * 512
                    dp = 64 * h  # den partition base for this half
                    probs = {}

                    def den_consume(m, dp=dp, probs=probs):
                        pr = probs[m]
                        for hi in range(2):
                            dr = dp + hi * 32
                            nc.tensor.matmul(
                                den[dr : dr + 1, :],
                                ones_sb[:],
                                pr[:, hi * 512 : (hi + 1) * 512],
                                start=(m == 0),
                                stop=(m == NM - 1),
                                tile_position=(0, dr),
                            )

                    def ctx_consume(m, ctx=ctx, g0=g0, probs=probs):
                        pr = probs[m]
                        for hi in range(2):
                            nc.tensor.matmul(
                                ctx[hi * 64 : (hi + 1) * 64, :],
                                vsb[:, m, (g0 + hi) * 64 : (g0 + hi + 1) * 64],
                                pr[:, hi * 512 : (hi + 1) * 512],
                                start=(m == 0),
                                stop=(m == NM - 1),
                            )

                    if first:
                        for kt in range(4):
                            vproj(kt)
                    for mp in range(0, NM, 2):
                        for m in (mp, mp + 1):
                            sc = psB.tile(
                                [128, 1024], F32, tag="sc", bufs=2,
                                name=f"sc_{qh}_{hp}_{h}_{m}",
                            )
                            # scores: both heads, row-tiled pair (bp0/bp64)
                            for hi in range(2):
                                po = hi * 64
                                nc.tensor.matmul(
                                    sc[:, hi * 512 : (hi + 1) * 512],
                                    kT[po : po + 64, hp, m * 128 : (m + 1) * 128],
                                    qT[po : po + 64, hp, q0 : q0 + 512],
                                    start=True,
                                    stop=True,
                                )
                            pr = pb.tile(
                                [128, 1024], CDT, tag="pr", bufs=8,
                                name=f"pr_{qh}_{hp}_{h}_{m}",
                            )
                            if m in DVE_SET or (last_h and m >= 14):
                                if last_h and m >= 14:
                                    # split: both engines finish the tile
                                    # fast so the tail chains unblock early
                                    nc.scalar.activation(
                                        pr[:, 0:512], sc[:, 0:512],
                                        EXP, scale=SCALE,
                                    )
                                    nc.vector.tensor_scalar(
                                        pr[:, 512:1024].bitcast(I16),
                                        sc[:, 512:1024], A16, B16, MULT, ADD,
                                    )
                                else:
                                    nc.vector.tensor_scalar(
                                        pr[:].bitcast(I16),
                                        sc[:], A16, B16, MULT, ADD,
                                    )
                            else:
                                nc.scalar.activation(
                                    pr[:], sc[:], EXP, scale=SCALE
                                )
                            probs[m] = pr
                        # previous h-iter's normalize in our first slots
                        if mp == 0 and deferred:
                            for cl in deferred:
                                cl()
                            deferred.clear()
                        # same-config runs of two: den pair-chain, ctx chain
                        if mp == 2:
                            den_consume(0)
                        elif mp == 4:
                            den_consume(1)
                            den_consume(2)
                            ctx_consume(0)
                        elif mp >= 6:
                            den_consume(mp - 3)
                            den_consume(mp - 2)
                            ctx_consume(mp - 5)
                            ctx_consume(mp - 4)
                            for j in (mp - 5, mp - 4):
                                probs.pop(j)
                            if last_h and mp == NM - 2:
                                # drain the tail as tightly as possible
                                den_consume(NM - 3)
                                ctx_consume(NM - 5)
                                ctx_consume(NM - 4)
                        # carried work: v-projection (first h-iter) feeds
                        # chains ahead of use; outproj drains wherever free
                        if first and 2 <= mp <= 12:
                            vproj(mp + 2)
                            vproj(mp + 3)
                        elif carried and mp >= 2:
                            carried.pop(0)()
                            if carried:
                                carried.pop(0)()
                    if first:
                        first = False
                    for j in range(NM - 3 + (1 if last_h else 0), NM):
                        den_consume(j)
                    for j in range(NM - 5 + (2 if last_h else 0), NM):
                        ctx_consume(j)
                    for j in range(NM - 5, NM):
                        probs.pop(j)

                    def normalize(qh=qh, hp=hp, h=h, ctx=ctx, den=den,
                                  dp=dp, q0=q0):
                        d33 = sm.tile(
                            [33, 512], F32, tag="d33", name=f"d33_{qh}_{hp}_{h}"
                        )
                        nc.vector.tensor_copy(d33[:], den[dp : dp + 33, :])
                        r33 = sm.tile(
                            [33, 512], F32, tag="r33", name=f"r33_{qh}_{hp}_{h}"
                        )
                        nc.vector.reciprocal_approx_fast(r33[:], d33[:])
                        rb = psB.tile(
                            [128, 512], F32, tag="acc512", bufs=1,
                            name=f"rb_{qh}_{hp}_{h}"
                        )
                        nc.tensor.matmul(
                            rb[:], sel33[:], r33[:], start=True, stop=True
                        )
                        rb_sb = sm.tile(
                            [128, 512], F32, tag="rbsb",
                            name=f"rbsb_{qh}_{hp}_{h}"
                        )
                        nc.scalar.copy(rb_sb[:], rb[:])
                        nc.vector.tensor_mul(
                            ctxT[:, hp, q0 : q0 + 512], ctx[:], rb_sb[:]
                        )
                        # outproj chunk lc=2qh+h complete once both hp groups
                        # normalized this half's columns
                        if hp == 1:
                            emit_outproj_lc(qh * 2 + h)

                    deferred.append(normalize)
        # tail: last h-iter's normalize, then remaining outproj work
        for cl in deferred:
            cl()
        deferred.clear()
        tags = ["acc512", "den", "ctx", "ctx"]
        for i, work in enumerate(carried):
            work.__defaults__ = (
                work.__defaults__[0],
                work.__defaults__[1],
                tags[i % 4],
            )
            work()
        carried.clear()


def make_in_maps(Q, K, V, Wq, bq, Wk, bk, Wv, bv, Wo, bo):
    Q = np.asarray(Q, np.float32)
    K = np.asarray(K, np.float32)
    V = np.asarray(V, np.float32)
    xqT = [np.ascontiguousarray(Q[b].T).astype(NP_CDT) for b in range(B)]
    xkT = [np.ascontiguousarray(K[b].T).astype(NP_CDT) for b in range(B)]
    xvT = [np.ascontiguousarray(V[b].T).astype(NP_CDT) for b in range(B)]
    in_maps = []
    for c in range(NCORES):
        b, g = divmod(c, HPC)
        fs = slice(g * FPC, (g + 1) * FPC)
        wqT = np.ascontiguousarray(np.asarray(Wq, np.float32)[fs, :].T).astype(NP_CDT)
        wkT = np.ascontiguousarray(np.asarray(Wk, np.float32)[fs, :].T).astype(NP_CDT)
        wvT = np.ascontiguousarray(np.asarray(Wv, np.float32)[fs, :].T).astype(NP_CDT)
        woT = np.ascontiguousarray(np.asarray(Wo, np.float32)[:, fs].T).astype(NP_CDT)
        bq2 = np.ascontiguousarray(
            np.asarray(bq, np.float32)[fs].reshape(NFT, 128).T
        )
        bk2 = np.ascontiguousarray(
            np.asarray(bk, np.float32)[fs].reshape(NFT, 128).T
        )
        bv_blk = np.asarray(bv, np.float32)[fs]
        in_maps.append(
            {
                "xqT": xqT[b],
                "xkT": xkT[b],
                "xvT": xvT[b],
                "wqT": wqT,
                "wkT": wkT,
                "wvT": wvT,
                "woT": woT,
                "bq2": bq2,
                "bk2": bk2,
                "bvb": np.broadcast_to(bv_blk, (128, FPC)).copy(),
            }
        )
    return in_maps


def assemble(results, bo):
    out = np.zeros((B, L, D), np.float32)
    for c in range(NCORES):
        b = c // HPC
        out[b] += results[c][OUT_NAME].T.astype(np.float32)
    out += np.asarray(bo, np.float32)[None, None, :]
    return out


def kernel(Q, K, V, Wq, bq, Wk, bk, Wv, bv, Wo, bo):
    if "nc" not in _CACHE:
        _CACHE["nc"] = build_nc()
    nc = _CACHE["nc"]
    in_maps = make_in_maps(Q, K, V, Wq, bq, Wk, bk, Wv, bv, Wo, bo)
    res = run_bass_kernel_spmd(nc, in_maps, core_ids=list(range(NCORES)))
    return assemble(res.results, bo)
